# revision 15
# baseline (speedup 1.0000x reference)
"""Causal self-attention (B=4, T=2048, C=1024, H=16) on 8 Trainium2 NeuronCores.

Sharding: core = (batch b = core//2, head-group g = core%2, 8 heads each).

v2 design (vs baseline): single globally-interleaved emission stream.
The ScalarE exp stream (~160 ACTIVATEs, ~150us busy) is roughly as large as
the attention-phase PE work, so a phase-separated schedule is ACT-bound in
the attention phase. Here the QKV projection and output projection matmuls
are injected as PE "filler" between the S->exp->PV steps of the attention
pipeline, so ACT always hides under a dense PE stream:

    QKV(0) | A(0)+QKV(1) | A(1)+QKV(2) | A(2)+QKV(3) | A(3)+proj(0,1,2) | proj(3)

All PE operands are bf16 (same 1 cycle/row as f32r, but FWL weight loads,
half the SBUF/DMA traffic, and no 4x penalty on <256-wide moving operands).
PSUM accumulation stays f32.  Per-core fp roundtrip keeps rel err ~1e-3,
well under the 2e-2 gate.

Attention per head pair (2 heads on disjoint 64-partition groups):
  S^T = K^T.T @ Q^T per 128-key tile (above-diagonal tiles skipped, diagonal
  tiles width-clipped), exp on ScalarE (PSUM->SBUF bf16), causal tril mask
  multiplied on DVE for diagonal tiles only, PV accumulates with a ones*pad
  column appended to V' so row 64 of the accumulator is the softmax
  denominator.  Normalize: DVE reciprocal of the denominator row (straight
  from PSUM), two 64-channel gpsimd partition-broadcasts, two DVE muls into
  y^T (bf16).
Host: transposes x per batch, slices/casts weights to bf16, sums the two
partials per batch and adds bproj.
"""

import os
import sys
from collections import deque

for _p in ("/opt/trn_rl_repo",):
    if _p not in sys.path:
        sys.path.append(_p)

import numpy as np

B, T, C = 4, 2048, 1024
H, D = 16, 64
HPC = 8          # heads per core
GC = HPC * D     # 512 channels per core
N_CORES = 8
P = 128
NT = T // 512    # 4  q-blocks of 512
MT = GC // 128   # 4  head pairs
CT = C // 128    # 8  contraction tiles
TT = T // 128    # 16 t-tiles of 128

_cached = {}


def _build():
    import concourse.tile as tile
    from concourse import bacc, mybir
    import concourse.bass as bass

    f32 = mybir.dt.float32
    bf16 = mybir.dt.bfloat16
    AF = mybir.ActivationFunctionType
    ADD = mybir.AluOpType.add
    MUL = mybir.AluOpType.mult

    nc = bacc.Bacc("TRN2", target_bir_lowering=False, debug=False)

    xT_d = nc.dram_tensor("xT", [C, T], bf16, kind="ExternalInput")
    wq_d = nc.dram_tensor("wq", [C, GC], bf16, kind="ExternalInput")
    wk_d = nc.dram_tensor("wk", [C, GC], bf16, kind="ExternalInput")
    wv_d = nc.dram_tensor("wv", [C, GC], bf16, kind="ExternalInput")
    bq_d = nc.dram_tensor("bq", [GC], f32, kind="ExternalInput")
    bk_d = nc.dram_tensor("bk", [GC], f32, kind="ExternalInput")
    bv_d = nc.dram_tensor("bv", [GC], f32, kind="ExternalInput")
    wp_d = nc.dram_tensor("wp", [GC, C], bf16, kind="ExternalInput")
    pad_d = nc.dram_tensor("pad", [T], f32, kind="ExternalInput")
    mask_d = nc.dram_tensor("mask", [P, 512], bf16, kind="ExternalInput")
    out_d = nc.dram_tensor("out", [T, C], f32, kind="ExternalOutput")
    out_r = out_d.rearrange("t (a n) -> t a n", a=2)

    with tile.TileContext(nc) as tc:
        with tc.tile_pool(name="persist", bufs=1) as persist, \
             tc.tile_pool(name="pspool", bufs=2, space="PSUM") as pspool, \
             tc.tile_pool(name="oopool", bufs=2, space="PSUM") as oopool, \
             tc.tile_pool(name="pppool", bufs=6) as pppool, \
             tc.tile_pool(name="ypool", bufs=4) as ypool, \
             tc.tile_pool(name="tpool", bufs=2) as tpool, \
             tc.tile_pool(name="bpool", bufs=2) as bpool, \
             tc.tile_pool(name="otpool", bufs=3) as otpool:

            QT = persist.tile([P, MT, T], bf16, tag="QT")
            KT = persist.tile([P, MT, T], bf16, tag="KT")
            Vp = persist.tile([P, TT, HPC, D + 1], bf16, tag="Vp")
            xt_all = persist.tile([P, NT, CT, 512], bf16, tag="xt")
            wq_s = persist.tile([P, CT, GC], bf16, tag="wq")
            wk_s = persist.tile([P, CT, GC], bf16, tag="wk")
            wv_s = persist.tile([P, CT, GC], bf16, tag="wv")
            wp_s = persist.tile([P, MT, C], bf16, tag="wp")
            pad_s = persist.tile([P, TT], f32, tag="pad")
            bq_s = persist.tile([P, MT], f32, tag="bq")
            bk_s = persist.tile([P, MT], f32, tag="bk")
            bv_s = persist.tile([P, GC], f32, tag="bv")
            tril_s = persist.tile([P, 512], bf16, tag="tril")

            xTr = xT_d.rearrange("(c p) t -> p c t", p=P)
            wqr = wq_d.rearrange("(c p) n -> p c n", p=P)
            wkr = wk_d.rearrange("(c p) n -> p c n", p=P)
            wvr = wv_d.rearrange("(c p) n -> p c n", p=P)

            # ---- startup DMAs, first-needed first; issue cost ~0.6us per
            # dma_start serializes on the issuing engine queue, so split
            # weights onto the (idle) Scalar HWDGE queue.
            nc.sync.dma_start(xt_all[:, 0, 0:4, :], xTr[:, 0:4, 0:512])
            nc.scalar.dma_start(wq_s[:, 0:4, :], wqr[:, 0:4, :])
            nc.sync.dma_start(xt_all[:, 0, 4:8, :], xTr[:, 4:8, 0:512])
            nc.scalar.dma_start(wq_s[:, 4:8, :], wqr[:, 4:8, :])
            for c4 in range(0, CT, 4):
                nc.scalar.dma_start(wk_s[:, c4:c4 + 4, :], wkr[:, c4:c4 + 4, :])
            for c4 in range(0, CT, 4):
                nc.scalar.dma_start(wv_s[:, c4:c4 + 4, :], wvr[:, c4:c4 + 4, :])
            nc.sync.dma_start(bq_s[:], bq_d.rearrange("(m p) -> p m", p=P))
            nc.sync.dma_start(bk_s[:], bk_d.rearrange("(m p) -> p m", p=P))
            nc.sync.dma_start(pad_s[:], pad_d.rearrange("(tt p) -> p tt", p=P))
            nc.sync.dma_start(tril_s[:], mask_d[:])
            bv_ap = bass.AP(tensor=bv_d[:].tensor, offset=0, ap=[[0, P], [1, GC]])
            nc.sync.dma_start(bv_s[:], bv_ap)
            for nt in range(1, NT):
                for c4 in range(0, CT, 4):
                    nc.sync.dma_start(xt_all[:, nt, c4:c4 + 4, :],
                                      xTr[:, c4:c4 + 4, nt * 512:(nt + 1) * 512])
            nc.scalar.dma_start(
                wp_s[:], wp_d.rearrange("(m p) n -> p m n", p=P))

            # Vp pad column: Vp[:, tt, h, 64] = pad[tt*128 + p] for all h
            for tt in range(TT):
                nc.vector.memset(Vp[:, tt, :, D:D + 1], 1.0)
                nc.vector.tensor_scalar(
                    out=Vp[:, tt, :, D:D + 1], in0=Vp[:, tt, :, D:D + 1],
                    scalar1=pad_s[:, tt:tt + 1], scalar2=None, op0=MUL)

            # ---- emission helpers --------------------------------------
            def qkv_gen(nt):
                """QKV projections for 512-token slice nt.

                Each yielded chunk is a SELF-CONTAINED 8-MM psum group
                (alloc -> 8 matmuls -> evac) so a filler burst never holds a
                pspool slot open across attention steps."""
                xt = xt_all[:, nt]
                for m in range(MT):
                    for W, qscale in ((wq_s, True), (wk_s, False)):
                        ps = pspool.tile([P, 512], f32, tag="SS", name="fps")
                        for c in range(CT):
                            nc.tensor.matmul(
                                ps[:], W[:, c, m * P:(m + 1) * P],
                                xt[:, c, :], start=(c == 0), stop=(c == CT - 1))
                        if qscale:
                            nc.vector.tensor_scalar(
                                out=QT[:, m, nt * 512:(nt + 1) * 512], in0=ps[:],
                                scalar1=bq_s[:, m:m + 1], scalar2=0.125,
                                op0=ADD, op1=MUL)
                        else:
                            nc.vector.tensor_scalar(
                                out=KT[:, m, nt * 512:(nt + 1) * 512], in0=ps[:],
                                scalar1=bk_s[:, m:m + 1], scalar2=None, op0=ADD)
                        yield 8
                for ts in range(4):
                    tt = nt * 4 + ts
                    ps = pspool.tile([P, 512], f32, tag="SS", name="fps")
                    for c in range(CT):
                        nc.tensor.matmul(
                            ps[:], xt[:, c, ts * P:(ts + 1) * P],
                            wv_s[:, c, :], start=(c == 0), stop=(c == CT - 1))
                    tmp = tpool.tile([P, GC], f32, tag="vtmp")
                    nc.vector.tensor_add(tmp[:], ps[:], bv_s[:])
                    nc.vector.tensor_scalar(
                        out=Vp[:, tt, :, 0:D],
                        in0=tmp[:].rearrange("p (h d) -> p h d", h=HPC),
                        scalar1=pad_s[:, tt:tt + 1], scalar2=None, op0=MUL)
                    yield 8

            yT_tiles = {}

            def proj_gen(qt_, defer_last=False):
                """Output projection for q-block qt_.

                Filler mode: self-contained 4-MM psum groups per (ts, nh).
                defer_last (tail): emit cj=0..2 for a pair of [P,2,512]
                groups before their cj=3 matmuls, so the tail PE stream
                doesn't queue behind the final pair's normalize chain."""
                yT_ = yT_tiles[qt_]
                if defer_last:
                    for tsp in range(2):
                        pss = []
                        for half in range(2):
                            ts = tsp * 2 + half
                            ps = pspool.tile([P, 2, 512], f32, tag="SS",
                                             name="pjps")
                            pss.append((ts, ps))
                            for nh in range(2):
                                for cj in range(MT - 1):
                                    nc.tensor.matmul(
                                        ps[:, nh, :],
                                        yT_[:, cj, ts * P:(ts + 1) * P],
                                        wp_s[:, cj, nh * 512:(nh + 1) * 512],
                                        start=(cj == 0), stop=False)
                                yield 3
                        for ts, ps in pss:
                            for nh in range(2):
                                nc.tensor.matmul(
                                    ps[:, nh, :],
                                    yT_[:, MT - 1, ts * P:(ts + 1) * P],
                                    wp_s[:, MT - 1, nh * 512:(nh + 1) * 512],
                                    start=False, stop=True)
                            yield 2
                            ot = otpool.tile([P, 2, 512], f32, tag="ot")
                            nc.vector.tensor_copy(ot[:], ps[:])
                            tt = qt_ * 4 + ts
                            nc.sync.dma_start(
                                out_r[tt * P:(tt + 1) * P, :, :], ot[:])
                    return
                for ts in range(4):
                    tt = qt_ * 4 + ts
                    for nh in range(2):
                        ps = pspool.tile([P, 512], f32, tag="SS", name="pjps")
                        for cj in range(MT):
                            nc.tensor.matmul(
                                ps[:], yT_[:, cj, ts * P:(ts + 1) * P],
                                wp_s[:, cj, nh * 512:(nh + 1) * 512],
                                start=(cj == 0), stop=(cj == MT - 1))
                        ot = otpool.tile([P, 512], f32, tag="ot2")
                        nc.vector.tensor_copy(ot[:], ps[:])
                        nc.sync.dma_start(
                            out_d[tt * P:(tt + 1) * P,
                                  nh * 512:(nh + 1) * 512], ot[:])
                        yield 4

            filler_q = deque()

            def emit_filler(n):
                budget = n
                while budget > 0 and filler_q:
                    try:
                        budget -= next(filler_q[0])
                    except StopIteration:
                        filler_q.popleft()

            def attention_block(qt, per_step):
                yTq = ypool.tile([P, MT, 512], bf16, tag="yT", name="yT")
                yT_tiles[qt] = yTq
                nk = 4 * (qt + 1)
                oo_map = {}
                pend = deque()
                LAG = 2
                per_step *= 2   # filler burst per 2-kt group

                def normalize(j):
                    OO_ = oo_map[j]
                    lraw = bpool.tile([1, 2, 512], f32, tag="lraw")
                    nc.vector.tensor_copy(lraw[0:1, :, :], OO_[D:D + 1, :, :])
                    lrec = bpool.tile([1, 2, 512], f32, tag="lrec")
                    nc.vector.reciprocal_approx_fast(
                        lrec[0:1, :, :], lraw[0:1, :, :])
                    bc = bpool.tile([P, 2, 512], f32, tag="bc")
                    nc.gpsimd.partition_broadcast(
                        bc[0:D, 0, :], lrec[0:1, 0, :], channels=D)
                    nc.gpsimd.partition_broadcast(
                        bc[0:D, 1, :], lrec[0:1, 1, :], channels=D)
                    nc.vector.tensor_mul(
                        yTq[0:D, j, :], OO_[0:D, 0, :], bc[0:D, 0, :])
                    nc.vector.tensor_mul(
                        yTq[D:P, j, :], OO_[0:D, 1, :], bc[0:D, 1, :])

                def emit_pv(entry):
                    j_, k_, z_, PP_ = entry
                    OO_ = oo_map[j_]
                    last = (k_ == nk - 1)
                    for e in range(2):
                        nc.tensor.matmul(
                            OO_[:, e, z_:512], Vp[:, k_, 2 * j_ + e, :],
                            PP_[:, e, z_:512],
                            start=(k_ == 0), stop=last)
                    if last:
                        normalize(j_)

                def emit_s(j, kt):
                    off = kt - 4 * qt
                    q0 = max(off, 0) * P
                    SS = pspool.tile([P, 2, 512], f32, tag="SS")
                    nc.tensor.matmul(
                        SS[:, 0, q0:512], KT[0:D, j, kt * P:(kt + 1) * P],
                        QT[0:D, j, qt * 512 + q0:(qt + 1) * 512],
                        start=True, stop=True)
                    nc.tensor.matmul(
                        SS[:, 1, q0:512], KT[D:P, j, kt * P:(kt + 1) * P],
                        QT[D:P, j, qt * 512 + q0:(qt + 1) * 512],
                        start=True, stop=True)
                    PP = pppool.tile([P, 2, 512], bf16, tag="PP")
                    nc.scalar.activation(
                        PP[:, :, q0:512], SS[:, :, q0:512], AF.Exp)
                    if off >= 0:
                        tm = tril_s[:, 0:512 - q0]
                        mask_b = bass.AP(
                            tensor=tm.tensor, offset=tm.offset,
                            ap=[list(tm.ap[0]), [0, 2], list(tm.ap[1])])
                        nc.vector.tensor_mul(
                            PP[:, :, q0:512], PP[:, :, q0:512], mask_b)
                    pend.append((j, kt, q0, PP))

                # double-step: [S,S]x2 then [PV,PV]x2 then one filler burst —
                # halves the category transitions (exposed LDWEIGHTS) and
                # keeps filler psum groups out of the S-tile slot rotation
                for j in range(MT):
                    oo_map[j] = oopool.tile([D + 1, 2, 512], f32, tag="OO",
                                            name="OO")
                    for g in range(0, nk, 2):
                        for kt in range(g, min(g + 2, nk)):
                            emit_s(j, kt)
                        while len(pend) > LAG:
                            emit_pv(pend.popleft())
                        emit_filler(per_step + (6 if g == 0 else 0))
                while pend:
                    emit_pv(pend.popleft())
                    emit_filler(3)

            # ---- schedule ----------------------------------------------
            for _ in qkv_gen(0):        # QKV(0) dense, no filler
                pass
            filler_q.append(qkv_gen(1))
            attention_block(0, 6)
            filler_q.append(qkv_gen(2))
            attention_block(1, 3)
            filler_q.append(qkv_gen(3))
            attention_block(2, 2)
            filler_q.append(proj_gen(0))
            filler_q.append(proj_gen(1))
            filler_q.append(proj_gen(2))
            attention_block(3, 2)
            emit_filler(10 ** 9)        # drain leftover filler
            for _ in proj_gen(3, defer_last=True):
                pass

    nc.compile()
    return nc


def _get_nc():
    if "nc" not in _cached:
        _cached["nc"] = _build()
    return _cached["nc"]


def kernel(x, padding_mask, Wqkv, bqkv, Wproj, bproj):
    from concourse.bass_utils import run_bass_kernel_spmd
    import ml_dtypes

    bf = ml_dtypes.bfloat16
    x = np.asarray(x, dtype=np.float32)
    padding_mask = np.asarray(padding_mask)
    Wqkv = np.asarray(Wqkv, dtype=np.float32)
    bqkv = np.asarray(bqkv, dtype=np.float32)
    Wproj = np.asarray(Wproj, dtype=np.float32)
    bproj = np.asarray(bproj, dtype=np.float32)
    assert x.shape == (B, T, C), x.shape

    nc = _get_nc()
    kk = np.arange(P)[:, None]
    qq = np.arange(512)[None, :]
    tril = (kk <= qq).astype(bf)

    in_maps = []
    for core in range(N_CORES):
        b, g = divmod(core, 2)
        sl = slice(g * GC, (g + 1) * GC)
        in_maps.append({
            "xT": np.ascontiguousarray(x[b].T).astype(bf),
            "wq": np.ascontiguousarray(Wqkv[:, 0 * C:1 * C][:, sl]).astype(bf),
            "wk": np.ascontiguousarray(Wqkv[:, 1 * C:2 * C][:, sl]).astype(bf),
            "wv": np.ascontiguousarray(Wqkv[:, 2 * C:3 * C][:, sl]).astype(bf),
            "bq": np.ascontiguousarray(bqkv[0 * C:1 * C][sl]),
            "bk": np.ascontiguousarray(bqkv[1 * C:2 * C][sl]),
            "bv": np.ascontiguousarray(bqkv[2 * C:3 * C][sl]),
            "wp": np.ascontiguousarray(Wproj[g * GC:(g + 1) * GC, :]).astype(bf),
            "pad": padding_mask[b].astype(np.float32),
            "mask": tril,
        })

    trace = bool(os.environ.get("BASS_KERNEL_TRACE"))
    res = run_bass_kernel_spmd(
        nc, in_maps, core_ids=list(range(N_CORES)), trace=trace)
    _cached["last_result"] = res

    out = np.empty((B, T, C), dtype=np.float32)
    for b in range(B):
        out[b] = res.results[2 * b]["out"] + res.results[2 * b + 1]["out"] + bproj
    return out


# revision 17
# speedup vs baseline: 1.0756x; 1.0756x over previous
"""Causal self-attention (B=4, T=2048, C=1024, H=16) on 8 Trainium2 NeuronCores.

Sharding: core = (batch b = core//2, head-group g = core%2, 8 heads each).

v2 design (vs baseline): single globally-interleaved emission stream.
The ScalarE exp stream (~160 ACTIVATEs, ~150us busy) is roughly as large as
the attention-phase PE work, so a phase-separated schedule is ACT-bound in
the attention phase. Here the QKV projection and output projection matmuls
are injected as PE "filler" between the S->exp->PV steps of the attention
pipeline, so ACT always hides under a dense PE stream:

    QKV(0) | A(0)+QKV(1) | A(1)+QKV(2) | A(2)+QKV(3) | A(3)+proj(0,1,2) | proj(3)

All PE operands are bf16 (same 1 cycle/row as f32r, but FWL weight loads,
half the SBUF/DMA traffic, and no 4x penalty on <256-wide moving operands).
PSUM accumulation stays f32.  Per-core fp roundtrip keeps rel err ~1e-3,
well under the 2e-2 gate.

Attention per head pair (2 heads on disjoint 64-partition groups):
  S^T = K^T.T @ Q^T per 128-key tile (above-diagonal tiles skipped, diagonal
  tiles width-clipped), exp on ScalarE (PSUM->SBUF bf16), causal tril mask
  multiplied on DVE for diagonal tiles only, PV accumulates with a ones*pad
  column appended to V' so row 64 of the accumulator is the softmax
  denominator.  Normalize: DVE reciprocal of the denominator row (straight
  from PSUM), two 64-channel gpsimd partition-broadcasts, two DVE muls into
  y^T (bf16).
Host: transposes x per batch, slices/casts weights to bf16, sums the two
partials per batch and adds bproj.
"""

import os
import sys
from collections import deque

for _p in ("/opt/trn_rl_repo",):
    if _p not in sys.path:
        sys.path.append(_p)

import numpy as np

B, T, C = 4, 2048, 1024
H, D = 16, 64
HPC = 8          # heads per core
GC = HPC * D     # 512 channels per core
N_CORES = 8
P = 128
NT = T // 512    # 4  q-blocks of 512
MT = GC // 128   # 4  head pairs
CT = C // 128    # 8  contraction tiles
TT = T // 128    # 16 t-tiles of 128

_cached = {}


def _build():
    import concourse.tile as tile
    from concourse import bacc, mybir
    import concourse.bass as bass

    f32 = mybir.dt.float32
    bf16 = mybir.dt.bfloat16
    AF = mybir.ActivationFunctionType
    ADD = mybir.AluOpType.add
    MUL = mybir.AluOpType.mult

    nc = bacc.Bacc("TRN2", target_bir_lowering=False, debug=False)

    xT_d = nc.dram_tensor("xT", [C, T], bf16, kind="ExternalInput")
    wq_d = nc.dram_tensor("wq", [C, GC], bf16, kind="ExternalInput")
    wk_d = nc.dram_tensor("wk", [C, GC], bf16, kind="ExternalInput")
    wv_d = nc.dram_tensor("wv", [C, GC], bf16, kind="ExternalInput")
    bq_d = nc.dram_tensor("bq", [GC], f32, kind="ExternalInput")
    bk_d = nc.dram_tensor("bk", [GC], f32, kind="ExternalInput")
    bv_d = nc.dram_tensor("bv", [GC], f32, kind="ExternalInput")
    wp_d = nc.dram_tensor("wp", [GC, C], bf16, kind="ExternalInput")
    pad_d = nc.dram_tensor("pad", [T], f32, kind="ExternalInput")
    mask_d = nc.dram_tensor("mask", [P, 512], bf16, kind="ExternalInput")
    out_d = nc.dram_tensor("out", [T, C], f32, kind="ExternalOutput")
    out_r = out_d.rearrange("t (a n) -> t a n", a=2)

    with tile.TileContext(nc) as tc:
        with tc.tile_pool(name="persist", bufs=1) as persist, \
             tc.tile_pool(name="pspool", bufs=2, space="PSUM") as pspool, \
             tc.tile_pool(name="oopool", bufs=1, space="PSUM") as oopool, \
             tc.tile_pool(name="fpool", bufs=2, space="PSUM") as fpool, \
             tc.tile_pool(name="pppool", bufs=8) as pppool, \
             tc.tile_pool(name="ypool", bufs=4) as ypool, \
             tc.tile_pool(name="tpool", bufs=2) as tpool, \
             tc.tile_pool(name="bpool", bufs=2) as bpool, \
             tc.tile_pool(name="otpool", bufs=3) as otpool:

            QT = persist.tile([P, MT, T], bf16, tag="QT")
            KT = persist.tile([P, MT, T], bf16, tag="KT")
            Vp = persist.tile([P, TT, HPC, D + 1], bf16, tag="Vp")
            xt_all = persist.tile([P, NT, CT, 512], bf16, tag="xt")
            wq_s = persist.tile([P, CT, GC], bf16, tag="wq")
            wk_s = persist.tile([P, CT, GC], bf16, tag="wk")
            wv_s = persist.tile([P, CT, GC], bf16, tag="wv")
            wp_s = persist.tile([P, MT, C], bf16, tag="wp")
            pad_s = persist.tile([P, TT], f32, tag="pad")
            bq_s = persist.tile([P, MT], f32, tag="bq")
            bk_s = persist.tile([P, MT], f32, tag="bk")
            bv_s = persist.tile([P, GC], f32, tag="bv")
            tril_s = persist.tile([P, 512], bf16, tag="tril")

            xTr = xT_d.rearrange("(c p) t -> p c t", p=P)
            wqr = wq_d.rearrange("(c p) n -> p c n", p=P)
            wkr = wk_d.rearrange("(c p) n -> p c n", p=P)
            wvr = wv_d.rearrange("(c p) n -> p c n", p=P)

            # ---- startup DMAs, first-needed first; issue cost ~0.6us per
            # dma_start serializes on the issuing engine queue, so split
            # weights onto the (idle) Scalar HWDGE queue.
            nc.sync.dma_start(xt_all[:, 0, 0:4, :], xTr[:, 0:4, 0:512])
            nc.scalar.dma_start(wq_s[:, 0:4, :], wqr[:, 0:4, :])
            nc.sync.dma_start(xt_all[:, 0, 4:8, :], xTr[:, 4:8, 0:512])
            nc.scalar.dma_start(wq_s[:, 4:8, :], wqr[:, 4:8, :])
            for c4 in range(0, CT, 4):
                nc.scalar.dma_start(wk_s[:, c4:c4 + 4, :], wkr[:, c4:c4 + 4, :])
            for c4 in range(0, CT, 4):
                nc.scalar.dma_start(wv_s[:, c4:c4 + 4, :], wvr[:, c4:c4 + 4, :])
            nc.sync.dma_start(bq_s[:], bq_d.rearrange("(m p) -> p m", p=P))
            nc.sync.dma_start(bk_s[:], bk_d.rearrange("(m p) -> p m", p=P))
            nc.sync.dma_start(pad_s[:], pad_d.rearrange("(tt p) -> p tt", p=P))
            nc.sync.dma_start(tril_s[:], mask_d[:])
            bv_ap = bass.AP(tensor=bv_d[:].tensor, offset=0, ap=[[0, P], [1, GC]])
            nc.sync.dma_start(bv_s[:], bv_ap)
            for nt in range(1, NT):
                for c4 in range(0, CT, 4):
                    nc.sync.dma_start(xt_all[:, nt, c4:c4 + 4, :],
                                      xTr[:, c4:c4 + 4, nt * 512:(nt + 1) * 512])
            nc.scalar.dma_start(
                wp_s[:], wp_d.rearrange("(m p) n -> p m n", p=P))

            # Vp pad column: Vp[:, tt, h, 64] = pad[tt*128 + p] for all h
            for tt in range(TT):
                nc.vector.memset(Vp[:, tt, :, D:D + 1], 1.0)
                nc.vector.tensor_scalar(
                    out=Vp[:, tt, :, D:D + 1], in0=Vp[:, tt, :, D:D + 1],
                    scalar1=pad_s[:, tt:tt + 1], scalar2=None, op0=MUL)

            # ---- emission helpers --------------------------------------
            def qkv_gen(nt):
                """QKV projections for 512-token slice nt.

                Each yielded chunk is a SELF-CONTAINED 8-MM psum group
                (alloc -> 8 matmuls -> evac) so a filler burst never holds a
                pspool slot open across attention steps."""
                xt = xt_all[:, nt]
                for m in range(MT):
                    for W, qscale in ((wq_s, True), (wk_s, False)):
                        ps = fpool.tile([P, 512], f32, tag="FP", name="fps")
                        for c in range(CT):
                            nc.tensor.matmul(
                                ps[:], W[:, c, m * P:(m + 1) * P],
                                xt[:, c, :], start=(c == 0), stop=(c == CT - 1))
                        if qscale:
                            nc.vector.tensor_scalar(
                                out=QT[:, m, nt * 512:(nt + 1) * 512], in0=ps[:],
                                scalar1=bq_s[:, m:m + 1], scalar2=0.125,
                                op0=ADD, op1=MUL)
                        else:
                            nc.vector.tensor_scalar(
                                out=KT[:, m, nt * 512:(nt + 1) * 512], in0=ps[:],
                                scalar1=bk_s[:, m:m + 1], scalar2=None, op0=ADD)
                        yield 8
                for ts in range(4):
                    tt = nt * 4 + ts
                    ps = fpool.tile([P, 512], f32, tag="FP", name="fps")
                    for c in range(CT):
                        nc.tensor.matmul(
                            ps[:], xt[:, c, ts * P:(ts + 1) * P],
                            wv_s[:, c, :], start=(c == 0), stop=(c == CT - 1))
                    tmp = tpool.tile([P, GC], f32, tag="vtmp")
                    nc.vector.tensor_add(tmp[:], ps[:], bv_s[:])
                    nc.vector.tensor_scalar(
                        out=Vp[:, tt, :, 0:D],
                        in0=tmp[:].rearrange("p (h d) -> p h d", h=HPC),
                        scalar1=pad_s[:, tt:tt + 1], scalar2=None, op0=MUL)
                    yield 8

            yT_tiles = {}

            def proj_gen(qt_, defer_last=False):
                """Output projection for q-block qt_.

                Filler mode: self-contained 4-MM psum groups per (ts, nh).
                defer_last (tail): emit cj=0..2 for a pair of [P,2,512]
                groups before their cj=3 matmuls, so the tail PE stream
                doesn't queue behind the final pair's normalize chain."""
                yT_ = yT_tiles[qt_]
                if defer_last:
                    for tsp in range(2):
                        pss = []
                        for half in range(2):
                            ts = tsp * 2 + half
                            ps = pspool.tile([P, 2, 512], f32, tag="SS",
                                             name="pjps")
                            pss.append((ts, ps))
                            for nh in range(2):
                                for cj in range(MT - 1):
                                    nc.tensor.matmul(
                                        ps[:, nh, :],
                                        yT_[:, cj, ts * P:(ts + 1) * P],
                                        wp_s[:, cj, nh * 512:(nh + 1) * 512],
                                        start=(cj == 0), stop=False)
                                yield 3
                        for ts, ps in pss:
                            for nh in range(2):
                                nc.tensor.matmul(
                                    ps[:, nh, :],
                                    yT_[:, MT - 1, ts * P:(ts + 1) * P],
                                    wp_s[:, MT - 1, nh * 512:(nh + 1) * 512],
                                    start=False, stop=True)
                            yield 2
                            ot = otpool.tile([P, 2, 512], f32, tag="ot")
                            nc.vector.tensor_copy(ot[:], ps[:])
                            tt = qt_ * 4 + ts
                            nc.sync.dma_start(
                                out_r[tt * P:(tt + 1) * P, :, :], ot[:])
                    return
                for ts in range(4):
                    tt = qt_ * 4 + ts
                    for nh in range(2):
                        ps = fpool.tile([P, 512], f32, tag="FP", name="pjps")
                        for cj in range(MT):
                            nc.tensor.matmul(
                                ps[:], yT_[:, cj, ts * P:(ts + 1) * P],
                                wp_s[:, cj, nh * 512:(nh + 1) * 512],
                                start=(cj == 0), stop=(cj == MT - 1))
                        ot = otpool.tile([P, 512], f32, tag="ot2")
                        nc.vector.tensor_copy(ot[:], ps[:])
                        nc.sync.dma_start(
                            out_d[tt * P:(tt + 1) * P,
                                  nh * 512:(nh + 1) * 512], ot[:])
                        yield 4

            filler_q = deque()

            def emit_filler(n):
                budget = n
                while budget > 0 and filler_q:
                    try:
                        budget -= next(filler_q[0])
                    except StopIteration:
                        filler_q.popleft()

            def attention_block(qt, per_step):
                yTq = ypool.tile([P, MT, 512], bf16, tag="yT", name="yT")
                yT_tiles[qt] = yTq
                nk = 4 * (qt + 1)
                oo_map = {}
                pend = deque()
                LAG = 4
                per_step *= 2   # filler burst per 2-kt group

                def normalize(j):
                    OO_ = oo_map[j]
                    lraw = bpool.tile([1, 2, 512], f32, tag="lraw")
                    nc.vector.tensor_copy(lraw[0:1, :, :], OO_[D:D + 1, :, :])
                    lrec = bpool.tile([1, 2, 512], f32, tag="lrec")
                    nc.vector.reciprocal_approx_fast(
                        lrec[0:1, :, :], lraw[0:1, :, :])
                    bc = bpool.tile([P, 2, 512], f32, tag="bc")
                    nc.gpsimd.partition_broadcast(
                        bc[0:D, 0, :], lrec[0:1, 0, :], channels=D)
                    nc.gpsimd.partition_broadcast(
                        bc[0:D, 1, :], lrec[0:1, 1, :], channels=D)
                    nc.vector.tensor_mul(
                        yTq[0:D, j, :], OO_[0:D, 0, :], bc[0:D, 0, :])
                    nc.vector.tensor_mul(
                        yTq[D:P, j, :], OO_[0:D, 1, :], bc[0:D, 1, :])

                def emit_pv(entry):
                    j_, k_, z_, PP_ = entry
                    OO_ = oo_map[j_]
                    last = (k_ == nk - 1)
                    for e in range(2):
                        nc.tensor.matmul(
                            OO_[:, e, z_:512], Vp[:, k_, 2 * j_ + e, :],
                            PP_[:, e, z_:512],
                            start=(k_ == 0), stop=last)
                    if last:
                        normalize(j_)

                def emit_s(j, kt):
                    off = kt - 4 * qt
                    q0 = max(off, 0) * P
                    SS = pspool.tile([P, 2, 512], f32, tag="SS")
                    nc.tensor.matmul(
                        SS[:, 0, q0:512], KT[0:D, j, kt * P:(kt + 1) * P],
                        QT[0:D, j, qt * 512 + q0:(qt + 1) * 512],
                        start=True, stop=True)
                    nc.tensor.matmul(
                        SS[:, 1, q0:512], KT[D:P, j, kt * P:(kt + 1) * P],
                        QT[D:P, j, qt * 512 + q0:(qt + 1) * 512],
                        start=True, stop=True)
                    PP = pppool.tile([P, 2, 512], bf16, tag="PP")
                    nc.scalar.activation(
                        PP[:, :, q0:512], SS[:, :, q0:512], AF.Exp)
                    if off >= 0:
                        tm = tril_s[:, 0:512 - q0]
                        mask_b = bass.AP(
                            tensor=tm.tensor, offset=tm.offset,
                            ap=[list(tm.ap[0]), [0, 2], list(tm.ap[1])])
                        nc.vector.tensor_mul(
                            PP[:, :, q0:512], PP[:, :, q0:512], mask_b)
                    pend.append((j, kt, q0, PP))

                # double-step: [S,S]x2 then [PV,PV]x2 then one filler burst —
                # halves the category transitions (exposed LDWEIGHTS) and
                # keeps filler psum groups out of the S-tile slot rotation
                for j in range(MT):
                    oo_map[j] = oopool.tile([D + 1, 2, 512], f32, tag="OO",
                                            name="OO")
                    for g in range(0, nk, 2):
                        for kt in range(g, min(g + 2, nk)):
                            emit_s(j, kt)
                        while len(pend) > LAG:
                            emit_pv(pend.popleft())
                        emit_filler(per_step + (6 if g == 0 else 0))
                while pend:
                    emit_pv(pend.popleft())
                    emit_filler(3)

            # ---- schedule ----------------------------------------------
            for _ in qkv_gen(0):        # QKV(0) dense, no filler
                pass
            filler_q.append(qkv_gen(1))
            attention_block(0, 6)
            filler_q.append(qkv_gen(2))
            attention_block(1, 3)
            filler_q.append(qkv_gen(3))
            attention_block(2, 2)
            filler_q.append(proj_gen(0))
            filler_q.append(proj_gen(1))
            filler_q.append(proj_gen(2))
            attention_block(3, 2)
            emit_filler(10 ** 9)        # drain leftover filler
            for _ in proj_gen(3, defer_last=True):
                pass

    nc.compile()
    return nc


def _get_nc():
    if "nc" not in _cached:
        _cached["nc"] = _build()
    return _cached["nc"]


def kernel(x, padding_mask, Wqkv, bqkv, Wproj, bproj):
    from concourse.bass_utils import run_bass_kernel_spmd
    import ml_dtypes

    bf = ml_dtypes.bfloat16
    x = np.asarray(x, dtype=np.float32)
    padding_mask = np.asarray(padding_mask)
    Wqkv = np.asarray(Wqkv, dtype=np.float32)
    bqkv = np.asarray(bqkv, dtype=np.float32)
    Wproj = np.asarray(Wproj, dtype=np.float32)
    bproj = np.asarray(bproj, dtype=np.float32)
    assert x.shape == (B, T, C), x.shape

    nc = _get_nc()
    kk = np.arange(P)[:, None]
    qq = np.arange(512)[None, :]
    tril = (kk <= qq).astype(bf)

    in_maps = []
    for core in range(N_CORES):
        b, g = divmod(core, 2)
        sl = slice(g * GC, (g + 1) * GC)
        in_maps.append({
            "xT": np.ascontiguousarray(x[b].T).astype(bf),
            "wq": np.ascontiguousarray(Wqkv[:, 0 * C:1 * C][:, sl]).astype(bf),
            "wk": np.ascontiguousarray(Wqkv[:, 1 * C:2 * C][:, sl]).astype(bf),
            "wv": np.ascontiguousarray(Wqkv[:, 2 * C:3 * C][:, sl]).astype(bf),
            "bq": np.ascontiguousarray(bqkv[0 * C:1 * C][sl]),
            "bk": np.ascontiguousarray(bqkv[1 * C:2 * C][sl]),
            "bv": np.ascontiguousarray(bqkv[2 * C:3 * C][sl]),
            "wp": np.ascontiguousarray(Wproj[g * GC:(g + 1) * GC, :]).astype(bf),
            "pad": padding_mask[b].astype(np.float32),
            "mask": tril,
        })

    trace = bool(os.environ.get("BASS_KERNEL_TRACE"))
    res = run_bass_kernel_spmd(
        nc, in_maps, core_ids=list(range(N_CORES)), trace=trace)
    _cached["last_result"] = res

    out = np.empty((B, T, C), dtype=np.float32)
    for b in range(B):
        out[b] = res.results[2 * b]["out"] + res.results[2 * b + 1]["out"] + bproj
    return out
